# revision 14
# baseline (speedup 1.0000x reference)
"""COPNLL loss kernel for Trainium2 (8 NeuronCores) — v2.

Math: V = (sig2e*I + s0*Z0 Z0^T + s1*Z1 Z1^T)/sig2 with Z0 (4096x1000),
Z1 (4096x500) one-hot. Woodbury reduces logdet(V) and m^T V^-1 m to the
500x500 Schur complement S = D1 - C^T diag(1/A) C with C = Z0^T Z1,
A = sig2e/s0 + counts0, D1 = sig2e/s1*I + diag(counts1).

Device plan (SPMD on 8 cores), column-sharded phase A:
  Each core processes ALL 4096 rows but only its 125 level-0 groups.
  Per 128-row chunk, ONE accumulating matmul with
    stationary st = [1 | m | r | oh0_125]  (exactly 128 cols)
    moving     rh = [oh1_500 | 1 | m | r]  (503 cols)
  yields C_block, counts0_block, a_block (rows 3:128) AND the global
  counts1, b, N, sum m, mtm, sum r^2 (rows 0:3) — no reduction needed.
  The ONLY collective is a single 64KB/rank int8 AllGather of
  [C_block int8 | counts0,a bitcast-f32]; counts1/b/mtm/r2 are computed
  fully locally on every core. A tiny warm-up AllGather posted first
  absorbs the CC-stream entry barrier during compute.
Phase C (redundant on all cores): upper-triangle S assembly (the t-vector
  rides as an appended moving column), block LDL with 4-iter Newton-Schulz
  inverses (quadratic init), Chebyshev trace of log (deg 7, bf16 coeffs,
  c0 + pad correction folded into a host constant).
"""

import math
import sys
import types

import numpy as np

import concourse.bass as bass
import concourse.bacc as bacc
import concourse.mybir as mybir
from concourse.bass import ds, ts
from concourse.bass_utils import run_bass_kernel_spmd
from concourse.masks import make_identity
from concourse.tile import TileContext


def _ensure_axon_hooks():
    """bass_utils imports antenv.axon_hooks when tracing; this image's antenv
    lacks it. Provide a shim (with the real ctypes NTFF hook when available)
    so trace=True/BASS_TRACE never crashes the kernel."""
    try:
        import antenv.axon_hooks  # noqa: F401
        return
    except ImportError:
        pass
    try:
        import trn_agent_boot.trn_boot as tb
        hook = tb._ntff_profile_via_ctypes("/opt/axon/libaxon_pjrt.so")
    except Exception:
        hook = None
    mod = types.ModuleType("antenv.axon_hooks")
    mod._hook = hook
    mod.get_axon_ntff_profile_hook = lambda: mod._hook

    def _set(h):
        mod._hook = h

    mod.set_axon_ntff_profile_hook = _set
    sys.modules["antenv.axon_hooks"] = mod
    try:
        import antenv
        antenv.axon_hooks = mod
    except ImportError:
        pass
    try:
        import concourse.bass_utils as bu
        _orig_upload = bu.upload_artifacts

        def _safe_upload(tmpdir):
            try:
                return _orig_upload(tmpdir)
            except Exception:
                return f"local:{tmpdir}"

        bu.upload_artifacts = _safe_upload
    except Exception:
        pass


_ensure_axon_hooks()

N = 4096
NCORES = 8
NCH = 32                   # 128-row chunks (all rows on every core)
NGRP = 4                   # chunk groups for build/matmul overlap
GSZ = NCH // NGRP          # 8 chunks per group
Q0 = 1000
B0 = Q0 // NCORES          # 125 level-0 groups per core
Q1 = 500
FR = Q1 + 2                # rh matmul width: [Z1 | 1 | m]
PAYW = 512                 # payload row bytes (int8)
SP = 512                   # padded S size
NBLK = SP // 128           # 4
W3 = Q1 - 3 * 128          # 116: valid width of the last S block
PADV = 4.0                 # pad diagonal value (mid-spectrum)
LO, HI = 1.4, 18.0         # eigenvalue bounds for NS init + Chebyshev
NS_ITERS = 4
CHEB_DEG = 7
NCOEF = CHEB_DEG + 1
CLIP = 4.2648907939226017  # sqrt(2)*erfinv(1-2e-5)
WARMUP = True

F32 = mybir.dt.float32
BF16 = mybir.dt.bfloat16
I8 = mybir.dt.int8
I32 = mybir.dt.int32
AX = mybir.AxisListType
OP = mybir.AluOpType
ACT = mybir.ActivationFunctionType

AG_IN = 128 * PAYW             # int8 bytes per rank
AG_OUT = NCORES * AG_IN


def cheb_coeffs(lo=LO, hi=HI, deg=CHEB_DEG):
    K = 4000
    th = (np.arange(K) + 0.5) * np.pi / K
    xk = np.cos(th)
    fk = np.log((hi - lo) / 2.0 * xk + (hi + lo) / 2.0)
    cs = np.array([2.0 / K * np.sum(fk * np.cos(j * th)) for j in range(deg + 1)])
    cs[0] *= 0.5
    return cs


def ns_init_coeffs(lo=LO, hi=HI):
    # X0 = a*I + b*B: minimax linear init for NS (equioscillating residual)
    m = (lo + hi) / 2.0
    s = (hi - lo) / 2.0
    b = 1.0 / (s * s / 2.0 - m * m)
    a = -2.0 * b * m
    return a, b


def _diag_fill(nc, tile_ap, value):
    nc.gpsimd.memset(tile_ap, 0.0)
    nc.gpsimd.affine_select(out=tile_ap, in_=tile_ap, compare_op=OP.not_equal,
                            fill=value, base=0, pattern=[[-1, 128]],
                            channel_multiplier=1)


def build_module(n_cores=NCORES, warmup=WARMUP):
    nc = bacc.Bacc(num_devices=n_cores)
    pk_d = nc.declare_dram_parameter("packed", [128, 4 * NCH], F32,
                                     isOutput=False)
    cst_d = nc.declare_dram_parameter("consts", [16], F32, isOutput=False)
    chb_d = nc.declare_dram_parameter("chebc", [NCOEF], F32, isOutput=False)
    out_d = nc.declare_dram_parameter("out", [1, 1], F32, isOutput=True)

    ag_in = nc.dram_tensor("ag_in", [AG_IN], I8)
    ag_out = nc.dram_tensor("ag_out", [AG_OUT], I8, addr_space="Shared")
    warm_in = nc.dram_tensor("warm_in", [64], F32)
    warm_out = nc.dram_tensor("warm_out", [64 * n_cores], F32,
                              addr_space="Shared")
    rg = [list(range(n_cores))]

    with TileContext(nc) as tc, \
         tc.tile_pool(name="consts", bufs=1) as consts, \
         tc.tile_pool(name="work", bufs=1) as work:

        # ---- warm-up collective: absorbs the CC entry barrier early ----
        if warmup and n_cores > 1:
            nc.gpsimd.collective_compute(
                "AllGather", OP.bypass, replica_groups=rg,
                ins=[warm_in[:]], outs=[warm_out[:]],
            )

        # ---- constants ----
        ident = consts.tile([128, 128], F32, tag="ident")
        make_identity(nc, ident)
        identB16 = consts.tile([128, 128], BF16, tag="identB16")
        nc.vector.tensor_copy(identB16, ident)
        ones128 = consts.tile([128, 128], F32, tag="ones128")
        nc.vector.memset(ones128, 1.0)

        cst_row = consts.tile([1, 16], F32, tag="cst_row")
        nc.sync.dma_start(cst_row, cst_d[:].rearrange("(p x) -> p x", p=1))
        chb_row = consts.tile([1, NCOEF], F32, tag="chb_row")
        nc.sync.dma_start(chb_row, chb_d[:].rearrange("(p x) -> p x", p=1))
        cst = consts.tile([128, 16], F32, tag="cst")
        chbB = consts.tile([128, NCOEF], F32, tag="chbB")
        with tc.tile_pool(name="setup_ps", bufs=2,
                          space=bass.MemorySpace.PSUM) as gps0:
            ps_b = gps0.tile([128, 16], F32, tag="gps0")
            nc.tensor.matmul(ps_b, ones128[0:1, :], cst_row,
                             start=True, stop=True)
            nc.vector.tensor_copy(cst, ps_b)
            ps_c = gps0.tile([128, NCOEF], F32, tag="gps0")
            nc.tensor.matmul(ps_c, ones128[0:1, :], chb_row,
                             start=True, stop=True)
            nc.vector.tensor_copy(chbB, ps_c)

        # iotas for the one-hot compares
        iota0i = work.tile([128, B0], I32, tag="iota0i")
        nc.gpsimd.iota(iota0i, pattern=[[1, B0]], base=0, channel_multiplier=0)
        iota0 = work.tile([128, B0], F32, tag="iota0")
        nc.vector.tensor_copy(iota0, iota0i)
        iota1i = work.tile([128, Q1], I32, tag="iota1i")
        nc.gpsimd.iota(iota1i, pattern=[[1, Q1]], base=0, channel_multiplier=0)
        iota1 = work.tile([128, Q1], F32, tag="iota1")
        nc.vector.tensor_copy(iota1, iota1i)
        # partition index (for pad masks on partitions 0..2)
        iotaPi = work.tile([128, 1], I32, tag="iotaPi")
        nc.gpsimd.iota(iotaPi, pattern=[[1, 1]], base=0, channel_multiplier=1)
        iotaP = work.tile([128, 1], F32, tag="iotaP")
        nc.vector.tensor_copy(iotaP, iotaPi)

        # ---- inputs -> m, resid ----
        packed = work.tile([128, 4 * NCH], F32, tag="packed")
        nc.sync.dma_start(packed, pk_d[:])
        yt = packed[:, 0:NCH]
        yp = packed[:, NCH:2 * NCH]
        idx0 = work.tile([128, NCH], F32, tag="idx0")
        nc.vector.tensor_copy(idx0, packed[:, 2 * NCH:3 * NCH].bitcast(I32))
        # idx0 shifted into this core's block: idx0s = idx0 - 125*core
        nc.vector.tensor_scalar(out=idx0, in0=idx0, scalar1=cst[:, 9:10],
                                scalar2=None, op0=OP.add)
        idx1 = work.tile([128, NCH], F32, tag="idx1")
        nc.vector.tensor_copy(idx1, packed[:, 3 * NCH:4 * NCH].bitcast(I32))
        resid = work.tile([128, NCH], F32, tag="resid")
        nc.vector.tensor_sub(resid, yt, yp)
        mvec = work.tile([128, NCH], F32, tag="mvec")
        nc.vector.tensor_scalar(out=mvec, in0=resid, scalar1=cst[:, 0:1],
                                scalar2=cst[:, 1:2], op0=OP.mult, op1=OP.min)
        nc.vector.tensor_scalar(out=mvec, in0=mvec, scalar1=cst[:, 8:9],
                                scalar2=None, op0=OP.max)
        smalls_c = work.tile([128, 9], F32, tag="smalls_c")
        nc.vector.memset(smalls_c, 0.0)
        scrN = work.tile([128, NCH], F32, tag="scrN")
        nc.vector.tensor_mul(scrN, mvec, mvec)
        nc.vector.tensor_reduce(smalls_c[:, 7:8], scrN, AX.X, OP.add)
        nc.vector.tensor_mul(scrN, resid, resid)
        nc.vector.tensor_reduce(smalls_c[:, 8:9], scrN, AX.X, OP.add)

        # ---- phase A: ONE matmul per chunk, grouped for overlap ----
        # st cols: 0:125 = oh0 (aligned writes), 125 = ones, 126 = m, 127 = 0
        # rh cols: 0:500 = oh1 (aligned; chunk stride padded to 512),
        #          500 = ones, 501 = m
        # psC: rows 0:125 = C|counts0|a, row 125 = counts1|N|sum m,
        #      row 126 = b|sum m|mtm, row 127 = 0
        STg = [work.tile([128, GSZ, 128], BF16, tag=f"STg{g}", name=f"STg{g}")
               for g in range(NGRP)]
        RHg = [work.tile([128, GSZ, 512], BF16, tag=f"RHg{g}", name=f"RHg{g}")
               for g in range(NGRP)]
        for g in range(NGRP):
            sl = slice(g * GSZ, (g + 1) * GSZ)
            nc.gpsimd.memset(STg[g][:, :, 125], 1.0)
            nc.gpsimd.memset(STg[g][:, :, 127], 0.0)
            nc.vector.tensor_copy(STg[g][:, :, 126], mvec[:, sl])
            nc.gpsimd.memset(RHg[g][:, :, Q1], 1.0)
            nc.vector.tensor_copy(RHg[g][:, :, Q1 + 1], mvec[:, sl])
            for cc in range(GSZ):
                c = g * GSZ + cc
                nc.gpsimd.tensor_scalar(out=STg[g][:, cc, 0:B0], in0=iota0,
                                        scalar1=idx0[:, c:c + 1],
                                        scalar2=None, op0=OP.is_equal)
                nc.vector.tensor_scalar(out=RHg[g][:, cc, 0:Q1], in0=iota1,
                                        scalar1=idx1[:, c:c + 1],
                                        scalar2=None, op0=OP.is_equal)

        pay = work.tile([128, PAYW], I8, tag="pay")
        g1s = work.tile([128, Q1], F32, tag="g1s")
        cf32 = work.tile([128, Q1], F32, tag="cf32")
        with tc.tile_pool(name="phA_ps", bufs=1,
                          space=bass.MemorySpace.PSUM) as pps:
            psC = pps.tile([128, FR], F32, tag="psC")
            for g in range(NGRP):
                for cc in range(GSZ):
                    c = g * GSZ + cc
                    nc.tensor.matmul(psC, STg[g][:, cc, :],
                                     RHg[g][:, cc, 0:FR],
                                     start=(c == 0), stop=(c == NCH - 1))
            # extract: C block -> int8 payload; counts0|a bitcast as f32
            nc.vector.tensor_copy(pay[:, 0:Q1], psC[:, 0:Q1])
            nc.vector.tensor_copy(pay[:, Q1:Q1 + 8].bitcast(F32),
                                  psC[:, Q1:Q1 + 2])
            # counts1 | b live on partitions 125:127: bounce to f32 SBUF,
            # then partition-shift to 0:2 via SBUF->SBUF DMA
            nc.vector.tensor_copy(cf32, psC[:, 0:Q1])
        nc.sync.dma_start(g1s[0:2, :], cf32[125:127, :])

        # ---- the single collective: AllGather of [C | counts0 | a] ----
        nc.sync.dma_start(ag_in[:].rearrange("(p f) -> p f", p=128), pay)
        if n_cores > 1:
            nc.gpsimd.collective_compute(
                "AllGather", OP.bypass, replica_groups=rg,
                ins=[ag_in[:]], outs=[ag_out[:]],
            )
        else:
            nc.sync.dma_start(ag_out[:], ag_in[:])

        # ---- phase C constants (fill the collective wait) ----
        i2 = consts.tile([128, 128], F32, tag="i2")              # 2*I
        _diag_fill(nc, i2, 2.0)
        shiftI = consts.tile([128, 128], F32, tag="shiftI")      # Cheb shift
        _diag_fill(nc, shiftI, (HI + LO) / (HI - LO))
        nsAI = consts.tile([128, 128], F32, tag="nsAI")          # NS init aI
        _diag_fill(nc, nsAI, ns_init_coeffs()[0])
        cIh = []
        for j in range(1, NCOEF):
            th_ = work.tile([128, 128], BF16, tag=f"cIh{j}", name=f"cIh{j}")
            nc.vector.tensor_scalar_mul(th_, ident, chbB[:, j:j + 1])
            cIh.append(th_)

        # c1 (counts1) / b per S-block as partition vectors via PE transpose
        cbts = []
        dSs = []
        with tc.tile_pool(name="tr_ps", bufs=2,
                          space=bass.MemorySpace.PSUM) as tps:
            for i in range(NBLK):
                wi = 128 if i < NBLK - 1 else W3
                psT = tps.tile([128, 2], F32, tag="pst")
                nc.tensor.transpose(psT[:wi, :], g1s[0:2, ds(i * 128, wi)],
                                    ident[0:2, 0:2])
                cbt = work.tile([128, 2], F32, tag=f"cb{i}", name=f"cb{i}")
                nc.vector.memset(cbt, 0.0)
                nc.vector.tensor_copy(cbt[:wi, :], psT[:wi, :])
                cbts.append(cbt)
                dS = work.tile([128, 1], F32, tag=f"dS{i}", name=f"dS{i}")
                nc.vector.tensor_scalar(out=dS, in0=cbt[:, 0:1],
                                        scalar1=cst[:, 3:4],
                                        scalar2=None, op0=OP.add)
                if i == NBLK - 1:
                    pm3 = work.tile([128, 1], mybir.dt.uint32, tag="pm3")
                    nc.vector.tensor_scalar(out=pm3, in0=iotaP,
                                            scalar1=float(W3) - 0.5,
                                            scalar2=None, op0=OP.is_gt)
                    padv = work.tile([128, 1], F32, tag="padv")
                    nc.vector.memset(padv, PADV)
                    nc.vector.copy_predicated(dS, pm3, padv)
                dSs.append(dS)

        # ---- unpack the AllGather + S assembly, pipelined per tile ----
        # valid level-0 rows on partitions 0:125; pads 125:128 are killed by
        # zeroing Winv there (stationary Cw pad rows become exactly 0)
        pmaskP = work.tile([128, 1], mybir.dt.uint32, tag="pmaskP")
        nc.vector.tensor_scalar(out=pmaskP, in0=iotaP, scalar1=float(B0) - 0.5,
                                scalar2=None, op0=OP.is_gt)
        iotaP8 = work.tile([128, NCORES], F32, tag="iotaP8")
        nc.vector.tensor_scalar_mul(iotaP8, ones128[:, 0:NCORES], iotaP)
        pmask8 = work.tile([128, NCORES], mybir.dt.uint32, tag="pmask8")
        nc.vector.tensor_scalar(out=pmask8, in0=iotaP8, scalar1=float(B0) - 0.5,
                                scalar2=None, op0=OP.is_gt)
        zcol = work.tile([128, NCORES], F32, tag="zcol")
        nc.vector.memset(zcol, 0.0)

        CT8 = [work.tile([128, PAYW], I8, tag=f"CT8_{t % 2}", name=f"CT8_{t}")
               for t in range(NCORES)]
        G0 = work.tile([128, NCORES, 512], BF16, tag="G0")   # [C | a | pad]
        Cw = work.tile([128, NCORES, 512], BF16, tag="Cw")
        Av = work.tile([128, NCORES], F32, tag="Av")
        Winv = work.tile([128, NCORES], F32, tag="Winv")
        aAv = work.tile([128, NCORES], F32, tag="aAv")
        Srow = [work.tile([128, SP], BF16, tag=f"Srow{i}", name=f"Srow{i}")
                for i in range(NBLK)]
        zvec = [work.tile([128, 1], F32, tag=f"z{i}", name=f"z{i}")
                for i in range(NBLK)]
        for i in range(NBLK):
            nc.vector.memset(Srow[i], 0.0)
            nc.vector.memset(zvec[i], 0.0)

        with tc.tile_pool(name="sasm_ps", bufs=1,
                          space=bass.MemorySpace.PSUM) as sps:
            psS = [sps.tile([128, Q1 + 1 - 128 * i], F32, tag=f"psS{i}",
                            name=f"psS{i}") for i in range(NBLK)]
            for t in range(NCORES):
                slot = ag_out[t * AG_IN:(t + 1) * AG_IN]
                nc.sync.dma_start(CT8[t], slot.rearrange("(p f) -> p f", p=128))
                nc.vector.tensor_copy(G0[:, t, 0:Q1], CT8[t][:, 0:Q1])
                cnA = CT8[t][:, Q1:Q1 + 8].bitcast(F32)   # [128, 2]
                nc.vector.tensor_scalar(out=Av[:, t:t + 1], in0=cnA[:, 0:1],
                                        scalar1=cst[:, 2:3], scalar2=None,
                                        op0=OP.add)
                nc.vector.tensor_copy(aAv[:, t:t + 1], cnA[:, 1:2])
                nc.vector.tensor_copy(G0[:, t, Q1:Q1 + 1], aAv[:, t:t + 1])
                nc.vector.reciprocal(Winv[:, t:t + 1], Av[:, t:t + 1])
                nc.vector.copy_predicated(Winv[:, t:t + 1], pmaskP,
                                          zcol[:, 0:1])
                nc.vector.tensor_scalar_mul(Cw[:, t, 0:Q1], G0[:, t, 0:Q1],
                                            Winv[:, t:t + 1])
                for i in range(NBLK):
                    wi = 128 if i < NBLK - 1 else W3
                    mw = Q1 + 1 - 128 * i
                    nc.tensor.matmul(psS[i][:wi, :],
                                     Cw[:, t, ds(i * 128, wi)],
                                     G0[:, t, ds(i * 128, mw)],
                                     start=(t == 0), stop=(t == NCORES - 1))
            # S rows (upper triangle) + diagonal; z = b - C^T (a/A)
            for i in range(NBLK):
                wi = 128 if i < NBLK - 1 else W3
                vw = Q1 - 128 * i
                nc.vector.tensor_scalar_mul(Srow[i][:wi, ds(i * 128, vw)],
                                            psS[i][:wi, 0:vw], -1.0)
                dgblk = work.tile([128, 128], BF16, tag="dgblk")
                nc.vector.tensor_scalar_mul(dgblk, ident, dSs[i])
                nc.vector.tensor_add(Srow[i][:, ts(i, 128)],
                                     Srow[i][:, ts(i, 128)], dgblk)
                nc.vector.tensor_sub(zvec[i][:wi, :], cbts[i][:wi, 1:2],
                                     psS[i][:wi, vw:vw + 1])

        # pad fixes for logA / qa (partitions 125:128)
        nc.vector.copy_predicated(Av, pmask8, ones128[:, 0:NCORES])
        nc.vector.copy_predicated(aAv, pmask8, zcol)
        scr8 = work.tile([128, NCORES], F32, tag="scr8")
        logA = work.tile([128, 1], F32, tag="logA")
        nc.scalar.activation(scr8, Av, ACT.Ln, accum_out=logA)
        nc.vector.tensor_mul(scr8, aAv, aAv)
        nc.vector.tensor_mul(scr8, scr8, Winv)
        qa = work.tile([128, 1], F32, tag="qa")
        nc.vector.tensor_reduce(qa, scr8, AX.X, OP.add)

        # ---- block LDL: NS inverses + deferred Chebyshev traces ----
        ns_b = ns_init_coeffs()[1]
        Binv = [work.tile([128, 128], F32, tag=f"Binv{k}", name=f"Binv{k}")
                for k in range(NBLK)]
        Wk = [work.tile([128, SP - (k + 1) * 128], BF16, tag=f"Wk{k}",
                        name=f"Wk{k}") for k in range(NBLK - 1)]
        Wk32 = [work.tile([128, SP - (k + 1) * 128], F32, tag=f"Wk32_{k}",
                          name=f"Wk32_{k}") for k in range(NBLK - 1)]
        trc = work.tile([128, NBLK], F32, tag="trc")
        qtt = work.tile([128, NBLK], F32, tag="qtt")

        with (
            tc.tile_pool(name="ldl", bufs=4) as ldl,
            tc.tile_pool(name="ldl_ps", bufs=4, space=bass.MemorySpace.PSUM) as lps,
        ):
            def cheb_chain(k):
                # trace of log(Bk) via Chebyshev; R += c_j T_j on the PE
                Bk = Srow[k][:, ts(k, 128)]
                bh = ldl.tile([128, 128], BF16, tag=f"bh{k}", name=f"bh{k}")
                nc.vector.tensor_scalar_mul(bh, Bk, 2.0 / (HI - LO))
                nc.vector.tensor_sub(bh, bh, shiftI)
                b2 = ldl.tile([128, 128], BF16, tag=f"b2{k}", name=f"b2{k}")
                nc.vector.tensor_scalar_mul(b2, bh, 2.0)
                R = lps.tile([128, 128], F32, tag="Rps", bufs=4,
                             name=f"Rps{k}")
                nc.tensor.matmul(R, cIh[0], bh, start=True,
                                 stop=(CHEB_DEG == 1))
                tprev, tcur = identB16, bh
                for j in range(2, CHEB_DEG + 1):
                    psc = lps.tile([128, 128], F32, tag="lps")
                    nc.tensor.matmul(psc, b2, tcur, start=True, stop=True)
                    tnext = ldl.tile([128, 128], BF16, tag=f"chT{k}",
                                     name=f"chT{k}_{j}", bufs=3)
                    nc.vector.tensor_sub(tnext, psc, tprev)
                    nc.tensor.matmul(R, cIh[j - 1], tnext,
                                     start=False, stop=(j == CHEB_DEG))
                    tprev, tcur = tcur, tnext
                Rsb = ldl.tile([128, 128], F32, tag="Rsb")
                nc.vector.tensor_mul(Rsb, R, ident)   # keep diagonal only
                nc.vector.tensor_reduce(trc[:, k:k + 1], Rsb, AX.X, OP.add)

            cheb_chain(0)
            for k in range(NBLK):
                Bk = Srow[k][:, ts(k, 128)]
                # NS: X0 = aI + b*B; X <- (2I - X B) X, all iterates sym.
                X = ldl.tile([128, 128], BF16, tag="nsX")
                nc.vector.tensor_scalar_mul(X, Bk, ns_b)
                nc.vector.tensor_add(X, X, nsAI)
                psX = None
                for it in range(NS_ITERS):
                    psP = lps.tile([128, 128], F32, tag="lps")
                    nc.tensor.matmul(psP, X, Bk, start=True, stop=True)
                    Z = ldl.tile([128, 128], BF16, tag="nsZ")
                    nc.vector.tensor_sub(Z, i2, psP)
                    psX = lps.tile([128, 128], F32, tag="lps")
                    nc.tensor.matmul(psX, Z, X, start=True, stop=True)
                    X = ldl.tile([128, 128], BF16, tag="nsX")
                    nc.vector.tensor_copy(X, psX)
                nc.vector.tensor_copy(Binv[k], psX)
                trail = SP - (k + 1) * 128 if k < NBLK - 1 else 0
                if trail:
                    psW = lps.tile([128, 384], F32, tag="lps")
                    nc.tensor.matmul(psW[:, :trail], X,
                                     Srow[k][:, (k + 1) * 128:SP],
                                     start=True, stop=True)
                    nc.vector.tensor_copy(Wk[k], psW[:, :trail])
                    nc.vector.tensor_copy(Wk32[k], psW[:, :trail])
                    for i in range(k + 1, NBLK):
                        uw = SP - 128 * i
                        off = (i - k - 1) * 128
                        psu = lps.tile([128, 384], F32, tag="lps")
                        nc.tensor.matmul(psu[:, :uw], Srow[k][:, ts(i, 128)],
                                         Wk[k][:, ds(off, uw)],
                                         start=True, stop=True)
                        nc.vector.tensor_sub(Srow[i][:, ds(128 * i, uw)],
                                             Srow[i][:, ds(128 * i, uw)],
                                             psu[:, :uw])
                        if i == k + 1:
                            cheb_chain(i)

            # forward substitution: z_i -= (Wk[k] block i)^T z_k
            for k in range(NBLK - 1):
                for i in range(k + 1, NBLK):
                    psz = lps.tile([128, 1], F32, tag="lps")
                    off = (i - k - 1) * 128
                    nc.tensor.matmul(psz, Wk32[k][:, ds(off, 128)], zvec[k],
                                     start=True, stop=True)
                    nc.vector.tensor_sub(zvec[i], zvec[i], psz)
            # quad_t = sum_k z_k^T Binv_k z_k
            for k in range(NBLK):
                psq = lps.tile([128, 1], F32, tag="lps")
                nc.tensor.matmul(psq, Binv[k], zvec[k], start=True, stop=True)
                uk = ldl.tile([128, 1], F32, tag="uk")
                nc.vector.tensor_copy(uk, psq)
                nc.vector.tensor_mul(qtt[:, k:k + 1], zvec[k], uk)

        # ---- final scalar assembly ----
        qtr = work.tile([128, 1], F32, tag="qtr")
        nc.vector.tensor_reduce(qtr, qtt, AX.X, OP.add)
        nc.vector.tensor_copy(smalls_c[:, 0:1], logA)
        nc.vector.tensor_copy(smalls_c[:, 1:2], qa)
        nc.vector.tensor_copy(smalls_c[:, 2:3], qtr)
        nc.vector.tensor_copy(smalls_c[:, 3:3 + NBLK], trc)
        smalls = work.tile([1, 9], F32, tag="smalls")
        ldS = work.tile([1, 1], F32, tag="ldS")
        with tc.tile_pool(name="fin_ps", bufs=1,
                          space=bass.MemorySpace.PSUM) as gps2:
            ps_sm = gps2.tile([128, 9], F32, tag="gps2")
            nc.tensor.matmul(ps_sm[0:1, :], ones128[:, 0:1], smalls_c,
                             start=True, stop=True)
            nc.vector.tensor_copy(smalls, ps_sm[0:1, :])
        nc.vector.tensor_reduce(ldS, smalls[:, 3:3 + NBLK], AX.X, OP.add)
        mtm = smalls[:, 7:8]
        r2g = smalls[:, 8:9]

        fin = work.tile([1, 8], F32, tag="fin")
        # quadK = quad_a + quad_t
        nc.vector.tensor_add(fin[:, 0:1], smalls[:, 1:2], smalls[:, 2:3])
        # mVinvm = (sig2/sig2e) * (mtm - quadK)
        nc.vector.tensor_sub(fin[:, 1:2], mtm, fin[:, 0:1])
        nc.vector.tensor_scalar_mul(fin[:, 1:2], fin[:, 1:2], cst[0:1, 6:7])
        # logdetV = const (incl. c0*SP + pad corr) + sum log A + tr chains
        nc.vector.tensor_add(fin[:, 2:3], smalls[:, 0:1], ldS)
        nc.vector.tensor_scalar(out=fin[:, 2:3], in0=fin[:, 2:3],
                                scalar1=cst[0:1, 4:5], scalar2=None, op0=OP.add)
        # sum_log_pdf = const2 - sum_r2/(2 sig2)
        nc.vector.tensor_scalar(out=fin[:, 3:4], in0=r2g, scalar1=cst[0:1, 7:8],
                                scalar2=cst[0:1, 5:6], op0=OP.mult, op1=OP.add)
        # total = 0.5*(logdetV + mVinvm - mtm + sum_log_pdf)
        nc.vector.tensor_add(fin[:, 4:5], fin[:, 2:3], fin[:, 1:2])
        nc.vector.tensor_sub(fin[:, 4:5], fin[:, 4:5], mtm)
        nc.vector.tensor_add(fin[:, 4:5], fin[:, 4:5], fin[:, 3:4])
        nc.vector.tensor_scalar_mul(fin[:, 4:5], fin[:, 4:5], 0.5)

        nc.sync.dma_start(out_d[:], fin[:, 4:5])

    nc.finalize()
    return nc


def host_consts(sig2e, sig2bs, core):
    import ml_dtypes
    s0, s1 = float(sig2bs[0]), float(sig2bs[1])
    sig2e = float(sig2e)
    sig2 = sig2e + s0 + s1
    cs = cheb_coeffs()
    csb = cs.astype(ml_dtypes.bfloat16).astype(np.float64)
    # exact c0 for all SP rows + remove the 12 pad rows' full cheb-log value
    xpad = (2.0 * PADV - (HI + LO)) / (HI - LO)
    tj, tp, tc_ = 0.0, 1.0, xpad
    chebpad = csb[1] * xpad
    for j in range(2, CHEB_DEG + 1):
        tn = 2.0 * xpad * tc_ - tp
        chebpad += csb[j] * tn
        tp, tc_ = tc_, tn
    chebpad += cs[0]
    c = np.zeros(16, np.float32)
    c[0] = 1.0 / math.sqrt(sig2)
    c[1] = CLIP
    c[2] = sig2e / s0
    c[3] = sig2e / s1
    c[4] = ((N - Q0 - Q1) * math.log(sig2e) + Q0 * math.log(s0)
            + Q1 * math.log(s1) - N * math.log(sig2)
            + SP * cs[0] - (128 - W3) * chebpad)
    c[5] = -0.5 * N * math.log(2.0 * math.pi * sig2)
    c[6] = sig2 / sig2e
    c[7] = -1.0 / (2.0 * sig2)
    c[8] = -CLIP
    c[9] = -float(B0 * core)
    return c


_CACHE = {}


def _get_module(n_cores=NCORES):
    if n_cores not in _CACHE:
        _CACHE[n_cores] = build_module(n_cores)
    return _CACHE[n_cores]


def make_in_maps(inputs, n_cores=NCORES):
    import ml_dtypes
    y_true = np.asarray(inputs["y_true"], np.float32).reshape(N)
    y_pred = np.asarray(inputs["y_pred"], np.float32).reshape(N)
    zi0 = np.asarray(inputs["Z_idx0"]).astype(np.int32).reshape(N)
    zi1 = np.asarray(inputs["Z_idx1"]).astype(np.int32).reshape(N)
    pk = np.concatenate([
        y_true.reshape(NCH, 128).T,
        y_pred.reshape(NCH, 128).T,
        zi0.reshape(NCH, 128).T.view(np.float32),
        zi1.reshape(NCH, 128).T.view(np.float32),
    ], axis=1)
    pk = np.ascontiguousarray(pk)
    cs = cheb_coeffs()
    chebc = cs.astype(ml_dtypes.bfloat16).astype(np.float32)
    maps = []
    for i in range(n_cores):
        c = host_consts(np.asarray(inputs["sig2e"]),
                        np.asarray(inputs["sig2bs"], np.float64), i)
        maps.append({"packed": pk, "consts": c, "chebc": chebc})
    return maps


def kernel(**inputs):
    nc = _get_module(NCORES)
    maps = make_in_maps(inputs, NCORES)
    res = run_bass_kernel_spmd(nc, maps, list(range(NCORES)))
    out = np.asarray(res.results[0]["out"], np.float32).reshape(1, 1)
    return out


# revision 15
# speedup vs baseline: 1.2215x; 1.2215x over previous
"""COPNLL loss kernel for Trainium2 (8 NeuronCores) — v2.

Math: V = (sig2e*I + s0*Z0 Z0^T + s1*Z1 Z1^T)/sig2 with Z0 (4096x1000),
Z1 (4096x500) one-hot. Woodbury reduces logdet(V) and m^T V^-1 m to the
500x500 Schur complement S = D1 - C^T diag(1/A) C with C = Z0^T Z1,
A = sig2e/s0 + counts0, D1 = sig2e/s1*I + diag(counts1).

Device plan (SPMD on 8 cores), column-sharded phase A:
  Each core processes ALL 4096 rows but only its 125 level-0 groups.
  Per 128-row chunk, ONE accumulating matmul with
    stationary st = [1 | m | r | oh0_125]  (exactly 128 cols)
    moving     rh = [oh1_500 | 1 | m | r]  (503 cols)
  yields C_block, counts0_block, a_block (rows 3:128) AND the global
  counts1, b, N, sum m, mtm, sum r^2 (rows 0:3) — no reduction needed.
  The ONLY collective is a single 64KB/rank int8 AllGather of
  [C_block int8 | counts0,a bitcast-f32]; counts1/b/mtm/r2 are computed
  fully locally on every core. A tiny warm-up AllGather posted first
  absorbs the CC-stream entry barrier during compute.
Phase C (redundant on all cores): upper-triangle S assembly (the t-vector
  rides as an appended moving column), block LDL with 4-iter Newton-Schulz
  inverses (quadratic init), Chebyshev trace of log (deg 7, bf16 coeffs,
  c0 + pad correction folded into a host constant).
"""

import math
import sys
import types

import numpy as np

import concourse.bass as bass
import concourse.bacc as bacc
import concourse.mybir as mybir
from concourse.bass import ds, ts
from concourse.bass_utils import run_bass_kernel_spmd
from concourse.masks import make_identity
from concourse.tile import TileContext


def _ensure_axon_hooks():
    """bass_utils imports antenv.axon_hooks when tracing; this image's antenv
    lacks it. Provide a shim (with the real ctypes NTFF hook when available)
    so trace=True/BASS_TRACE never crashes the kernel."""
    try:
        import antenv.axon_hooks  # noqa: F401
        return
    except ImportError:
        pass
    try:
        import trn_agent_boot.trn_boot as tb
        hook = tb._ntff_profile_via_ctypes("/opt/axon/libaxon_pjrt.so")
    except Exception:
        hook = None
    mod = types.ModuleType("antenv.axon_hooks")
    mod._hook = hook
    mod.get_axon_ntff_profile_hook = lambda: mod._hook

    def _set(h):
        mod._hook = h

    mod.set_axon_ntff_profile_hook = _set
    sys.modules["antenv.axon_hooks"] = mod
    try:
        import antenv
        antenv.axon_hooks = mod
    except ImportError:
        pass
    try:
        import concourse.bass_utils as bu
        _orig_upload = bu.upload_artifacts

        def _safe_upload(tmpdir):
            try:
                return _orig_upload(tmpdir)
            except Exception:
                return f"local:{tmpdir}"

        bu.upload_artifacts = _safe_upload
    except Exception:
        pass


_ensure_axon_hooks()

N = 4096
NCORES = 8
NCH = 32                   # 128-row chunks (all rows on every core)
NGRP = 4                   # chunk groups for build/matmul overlap
GSZ = NCH // NGRP          # 8 chunks per group
Q0 = 1000
B0 = Q0 // NCORES          # 125 level-0 groups per core
Q1 = 500
FR = Q1 + 2                # rh matmul width: [Z1 | 1 | m]
PAYW = 512                 # payload row bytes (int8)
SP = 512                   # padded S size
NBLK = SP // 128           # 4
W3 = Q1 - 3 * 128          # 116: valid width of the last S block
PADV = 4.0                 # pad diagonal value (mid-spectrum)
LO, HI = 1.4, 18.0         # eigenvalue bounds for NS init + Chebyshev
NS_ITERS = 4
CHEB_DEG = 7
NCOEF = CHEB_DEG + 1
CLIP = 4.2648907939226017  # sqrt(2)*erfinv(1-2e-5)
WARMUP = True

F32 = mybir.dt.float32
BF16 = mybir.dt.bfloat16
I8 = mybir.dt.int8
I32 = mybir.dt.int32
AX = mybir.AxisListType
OP = mybir.AluOpType
ACT = mybir.ActivationFunctionType

AG_IN = 128 * PAYW             # int8 bytes per rank
AG_OUT = NCORES * AG_IN


def cheb_coeffs(lo=LO, hi=HI, deg=CHEB_DEG):
    K = 4000
    th = (np.arange(K) + 0.5) * np.pi / K
    xk = np.cos(th)
    fk = np.log((hi - lo) / 2.0 * xk + (hi + lo) / 2.0)
    cs = np.array([2.0 / K * np.sum(fk * np.cos(j * th)) for j in range(deg + 1)])
    cs[0] *= 0.5
    return cs


def ns_init_coeffs(lo=LO, hi=HI):
    # X0 = a*I + b*B: minimax linear init for NS (equioscillating residual)
    m = (lo + hi) / 2.0
    s = (hi - lo) / 2.0
    b = 1.0 / (s * s / 2.0 - m * m)
    a = -2.0 * b * m
    return a, b


def _diag_fill(nc, tile_ap, value):
    nc.gpsimd.memset(tile_ap, 0.0)
    nc.gpsimd.affine_select(out=tile_ap, in_=tile_ap, compare_op=OP.not_equal,
                            fill=value, base=0, pattern=[[-1, 128]],
                            channel_multiplier=1)


def build_module(n_cores=NCORES, warmup=WARMUP):
    nc = bacc.Bacc(num_devices=n_cores)
    pk_d = nc.declare_dram_parameter("packed", [128, 4 * NCH], F32,
                                     isOutput=False)
    cst_d = nc.declare_dram_parameter("consts", [16], F32, isOutput=False)
    chb_d = nc.declare_dram_parameter("chebc", [NCOEF], F32, isOutput=False)
    out_d = nc.declare_dram_parameter("out", [1, 1], F32, isOutput=True)

    ag_in = nc.dram_tensor("ag_in", [AG_IN], I8)
    ag_out = nc.dram_tensor("ag_out", [AG_OUT], I8, addr_space="Shared")
    warm_in = nc.dram_tensor("warm_in", [64], F32)
    warm_out = nc.dram_tensor("warm_out", [64 * n_cores], F32,
                              addr_space="Shared")
    rg = [list(range(n_cores))]

    with TileContext(nc) as tc, \
         tc.tile_pool(name="consts", bufs=1) as consts, \
         tc.tile_pool(name="work", bufs=1) as work:

        # ---- warm-up collective: absorbs the CC entry barrier early ----
        if warmup and n_cores > 1:
            nc.gpsimd.collective_compute(
                "AllGather", OP.bypass, replica_groups=rg,
                ins=[warm_in[:]], outs=[warm_out[:]],
            )

        # ---- constants ----
        ident = consts.tile([128, 128], F32, tag="ident")
        make_identity(nc, ident)
        identB16 = consts.tile([128, 128], BF16, tag="identB16")
        nc.vector.tensor_copy(identB16, ident)
        ones128 = consts.tile([128, 128], F32, tag="ones128")
        nc.vector.memset(ones128, 1.0)

        cst_row = consts.tile([1, 16], F32, tag="cst_row")
        nc.sync.dma_start(cst_row, cst_d[:].rearrange("(p x) -> p x", p=1))
        chb_row = consts.tile([1, NCOEF], F32, tag="chb_row")
        nc.sync.dma_start(chb_row, chb_d[:].rearrange("(p x) -> p x", p=1))
        cst = consts.tile([128, 16], F32, tag="cst")
        chbB = consts.tile([128, NCOEF], F32, tag="chbB")
        with tc.tile_pool(name="setup_ps", bufs=2,
                          space=bass.MemorySpace.PSUM) as gps0:
            ps_b = gps0.tile([128, 16], F32, tag="gps0")
            nc.tensor.matmul(ps_b, ones128[0:1, :], cst_row,
                             start=True, stop=True)
            nc.vector.tensor_copy(cst, ps_b)
            ps_c = gps0.tile([128, NCOEF], F32, tag="gps0")
            nc.tensor.matmul(ps_c, ones128[0:1, :], chb_row,
                             start=True, stop=True)
            nc.vector.tensor_copy(chbB, ps_c)

        # iotas for the one-hot compares
        iota0i = work.tile([128, B0], I32, tag="iota0i")
        nc.gpsimd.iota(iota0i, pattern=[[1, B0]], base=0, channel_multiplier=0)
        iota0 = work.tile([128, B0], F32, tag="iota0")
        nc.vector.tensor_copy(iota0, iota0i)
        iota1i = work.tile([128, Q1], I32, tag="iota1i")
        nc.gpsimd.iota(iota1i, pattern=[[1, Q1]], base=0, channel_multiplier=0)
        iota1 = work.tile([128, Q1], F32, tag="iota1")
        nc.vector.tensor_copy(iota1, iota1i)
        # partition index (for pad masks on partitions 0..2)
        iotaPi = work.tile([128, 1], I32, tag="iotaPi")
        nc.gpsimd.iota(iotaPi, pattern=[[1, 1]], base=0, channel_multiplier=1)
        iotaP = work.tile([128, 1], F32, tag="iotaP")
        nc.vector.tensor_copy(iotaP, iotaPi)

        # ---- inputs -> m, resid ----
        packed = work.tile([128, 4 * NCH], F32, tag="packed")
        nc.sync.dma_start(packed, pk_d[:])
        yt = packed[:, 0:NCH]
        yp = packed[:, NCH:2 * NCH]
        idx0 = work.tile([128, NCH], F32, tag="idx0")
        nc.vector.tensor_copy(idx0, packed[:, 2 * NCH:3 * NCH].bitcast(I32))
        # idx0 shifted into this core's block: idx0s = idx0 - 125*core
        nc.vector.tensor_scalar(out=idx0, in0=idx0, scalar1=cst[:, 9:10],
                                scalar2=None, op0=OP.add)
        idx1 = work.tile([128, NCH], F32, tag="idx1")
        nc.vector.tensor_copy(idx1, packed[:, 3 * NCH:4 * NCH].bitcast(I32))
        resid = work.tile([128, NCH], F32, tag="resid")
        nc.vector.tensor_sub(resid, yt, yp)
        mvec = work.tile([128, NCH], F32, tag="mvec")
        nc.vector.tensor_scalar(out=mvec, in0=resid, scalar1=cst[:, 0:1],
                                scalar2=cst[:, 1:2], op0=OP.mult, op1=OP.min)
        nc.vector.tensor_scalar(out=mvec, in0=mvec, scalar1=cst[:, 8:9],
                                scalar2=None, op0=OP.max)
        smalls_c = work.tile([128, 9], F32, tag="smalls_c")
        nc.vector.memset(smalls_c, 0.0)
        scrN = work.tile([128, NCH], F32, tag="scrN")
        nc.vector.tensor_mul(scrN, mvec, mvec)
        nc.vector.tensor_reduce(smalls_c[:, 7:8], scrN, AX.X, OP.add)
        nc.vector.tensor_mul(scrN, resid, resid)
        nc.vector.tensor_reduce(smalls_c[:, 8:9], scrN, AX.X, OP.add)

        # ---- phase A: ONE matmul per chunk, grouped for overlap ----
        # st cols: 0:125 = oh0 (aligned writes), 125 = ones, 126 = m, 127 = 0
        # rh cols: 0:500 = oh1 (aligned; chunk stride padded to 512),
        #          500 = ones, 501 = m
        # psC: rows 0:125 = C|counts0|a, row 125 = counts1|N|sum m,
        #      row 126 = b|sum m|mtm, row 127 = 0
        STg = [work.tile([128, GSZ, 128], BF16, tag=f"STg{g}", name=f"STg{g}")
               for g in range(NGRP)]
        RHg = [work.tile([128, GSZ, 512], BF16, tag=f"RHg{g}", name=f"RHg{g}")
               for g in range(NGRP)]
        for g in range(NGRP):
            sl = slice(g * GSZ, (g + 1) * GSZ)
            nc.gpsimd.memset(STg[g][:, :, 125], 1.0)
            nc.gpsimd.memset(STg[g][:, :, 127], 0.0)
            nc.vector.tensor_copy(STg[g][:, :, 126], mvec[:, sl])
            nc.gpsimd.memset(RHg[g][:, :, Q1], 1.0)
            nc.vector.tensor_copy(RHg[g][:, :, Q1 + 1], mvec[:, sl])
            for cc in range(GSZ):
                c = g * GSZ + cc
                nc.vector.tensor_scalar(out=STg[g][:, cc, 0:B0], in0=iota0,
                                        scalar1=idx0[:, c:c + 1],
                                        scalar2=None, op0=OP.is_equal)
                nc.vector.tensor_scalar(out=RHg[g][:, cc, 0:Q1], in0=iota1,
                                        scalar1=idx1[:, c:c + 1],
                                        scalar2=None, op0=OP.is_equal)

        pay = work.tile([128, PAYW], I8, tag="pay")
        g1s = work.tile([128, Q1], F32, tag="g1s")
        cf32 = work.tile([128, Q1], F32, tag="cf32")
        with tc.tile_pool(name="phA_ps", bufs=1,
                          space=bass.MemorySpace.PSUM) as pps:
            psC = pps.tile([128, FR], F32, tag="psC")
            for g in range(NGRP):
                for cc in range(GSZ):
                    c = g * GSZ + cc
                    nc.tensor.matmul(psC, STg[g][:, cc, :],
                                     RHg[g][:, cc, 0:FR],
                                     start=(c == 0), stop=(c == NCH - 1))
            # extract: C block -> int8 payload; counts0|a bitcast as f32
            nc.vector.tensor_copy(pay[:, 0:Q1], psC[:, 0:Q1])
            nc.vector.tensor_copy(pay[:, Q1:Q1 + 8].bitcast(F32),
                                  psC[:, Q1:Q1 + 2])
            # counts1 | b live on partitions 125:127: bounce to f32 SBUF,
            # then partition-shift to 0:2 via SBUF->SBUF DMA
            nc.vector.tensor_copy(cf32, psC[:, 0:Q1])
        nc.sync.dma_start(g1s[0:2, :], cf32[125:127, :])

        # ---- the single collective: AllGather of [C | counts0 | a] ----
        nc.sync.dma_start(ag_in[:].rearrange("(p f) -> p f", p=128), pay)
        if n_cores > 1:
            nc.gpsimd.collective_compute(
                "AllGather", OP.bypass, replica_groups=rg,
                ins=[ag_in[:]], outs=[ag_out[:]],
            )
        else:
            nc.sync.dma_start(ag_out[:], ag_in[:])

        # ---- phase C constants (fill the collective wait) ----
        i2 = consts.tile([128, 128], F32, tag="i2")              # 2*I
        _diag_fill(nc, i2, 2.0)
        shiftI = consts.tile([128, 128], F32, tag="shiftI")      # Cheb shift
        _diag_fill(nc, shiftI, (HI + LO) / (HI - LO))
        nsAI = consts.tile([128, 128], F32, tag="nsAI")          # NS init aI
        _diag_fill(nc, nsAI, ns_init_coeffs()[0])
        cIh = []
        for j in range(1, NCOEF):
            th_ = work.tile([128, 128], BF16, tag=f"cIh{j}", name=f"cIh{j}")
            nc.vector.tensor_scalar_mul(th_, ident, chbB[:, j:j + 1])
            cIh.append(th_)

        # c1 (counts1) / b per S-block as partition vectors via PE transpose
        cbts = []
        dSs = []
        with tc.tile_pool(name="tr_ps", bufs=2,
                          space=bass.MemorySpace.PSUM) as tps:
            for i in range(NBLK):
                wi = 128 if i < NBLK - 1 else W3
                psT = tps.tile([128, 2], F32, tag="pst")
                nc.tensor.transpose(psT[:wi, :], g1s[0:2, ds(i * 128, wi)],
                                    ident[0:2, 0:2])
                cbt = work.tile([128, 2], F32, tag=f"cb{i}", name=f"cb{i}")
                nc.vector.memset(cbt, 0.0)
                nc.vector.tensor_copy(cbt[:wi, :], psT[:wi, :])
                cbts.append(cbt)
                dS = work.tile([128, 1], F32, tag=f"dS{i}", name=f"dS{i}")
                nc.vector.tensor_scalar(out=dS, in0=cbt[:, 0:1],
                                        scalar1=cst[:, 3:4],
                                        scalar2=None, op0=OP.add)
                if i == NBLK - 1:
                    pm3 = work.tile([128, 1], mybir.dt.uint32, tag="pm3")
                    nc.vector.tensor_scalar(out=pm3, in0=iotaP,
                                            scalar1=float(W3) - 0.5,
                                            scalar2=None, op0=OP.is_gt)
                    padv = work.tile([128, 1], F32, tag="padv")
                    nc.vector.memset(padv, PADV)
                    nc.vector.copy_predicated(dS, pm3, padv)
                dSs.append(dS)

        # ---- unpack the AllGather + S assembly, pipelined per tile ----
        # valid level-0 rows on partitions 0:125; pads 125:128 are killed by
        # zeroing Winv there (stationary Cw pad rows become exactly 0)
        pmaskP = work.tile([128, 1], mybir.dt.uint32, tag="pmaskP")
        nc.vector.tensor_scalar(out=pmaskP, in0=iotaP, scalar1=float(B0) - 0.5,
                                scalar2=None, op0=OP.is_gt)
        iotaP8 = work.tile([128, NCORES], F32, tag="iotaP8")
        nc.vector.tensor_scalar_mul(iotaP8, ones128[:, 0:NCORES], iotaP)
        pmask8 = work.tile([128, NCORES], mybir.dt.uint32, tag="pmask8")
        nc.vector.tensor_scalar(out=pmask8, in0=iotaP8, scalar1=float(B0) - 0.5,
                                scalar2=None, op0=OP.is_gt)
        zcol = work.tile([128, NCORES], F32, tag="zcol")
        nc.vector.memset(zcol, 0.0)

        CT8 = [work.tile([128, PAYW], I8, tag=f"CT8_{t % 2}", name=f"CT8_{t}")
               for t in range(NCORES)]
        G0 = work.tile([128, NCORES, 512], BF16, tag="G0")   # [C | a | pad]
        Cw = work.tile([128, NCORES, 512], BF16, tag="Cw")
        Av = work.tile([128, NCORES], F32, tag="Av")
        Winv = work.tile([128, NCORES], F32, tag="Winv")
        aAv = work.tile([128, NCORES], F32, tag="aAv")
        Srow = [work.tile([128, SP], BF16, tag=f"Srow{i}", name=f"Srow{i}")
                for i in range(NBLK)]
        zvec = [work.tile([128, 1], F32, tag=f"z{i}", name=f"z{i}")
                for i in range(NBLK)]
        for i in range(NBLK):
            nc.vector.memset(Srow[i], 0.0)
            nc.vector.memset(zvec[i], 0.0)

        with tc.tile_pool(name="sasm_ps", bufs=1,
                          space=bass.MemorySpace.PSUM) as sps:
            psS = [sps.tile([128, Q1 + 1 - 128 * i], F32, tag=f"psS{i}",
                            name=f"psS{i}") for i in range(NBLK)]
            for t in range(NCORES):
                slot = ag_out[t * AG_IN:(t + 1) * AG_IN]
                nc.sync.dma_start(CT8[t], slot.rearrange("(p f) -> p f", p=128))
                nc.vector.tensor_copy(G0[:, t, 0:Q1], CT8[t][:, 0:Q1])
                cnA = CT8[t][:, Q1:Q1 + 8].bitcast(F32)   # [128, 2]
                nc.vector.tensor_scalar(out=Av[:, t:t + 1], in0=cnA[:, 0:1],
                                        scalar1=cst[:, 2:3], scalar2=None,
                                        op0=OP.add)
                nc.vector.tensor_copy(aAv[:, t:t + 1], cnA[:, 1:2])
                nc.vector.tensor_copy(G0[:, t, Q1:Q1 + 1], aAv[:, t:t + 1])
                nc.vector.reciprocal(Winv[:, t:t + 1], Av[:, t:t + 1])
                nc.vector.copy_predicated(Winv[:, t:t + 1], pmaskP,
                                          zcol[:, 0:1])
                nc.vector.tensor_scalar_mul(Cw[:, t, 0:Q1], G0[:, t, 0:Q1],
                                            Winv[:, t:t + 1])
                for i in range(NBLK):
                    wi = 128 if i < NBLK - 1 else W3
                    mw = Q1 + 1 - 128 * i
                    nc.tensor.matmul(psS[i][:wi, :],
                                     Cw[:, t, ds(i * 128, wi)],
                                     G0[:, t, ds(i * 128, mw)],
                                     start=(t == 0), stop=(t == NCORES - 1))
            # S rows (upper triangle) + diagonal; z = b - C^T (a/A)
            for i in range(NBLK):
                wi = 128 if i < NBLK - 1 else W3
                vw = Q1 - 128 * i
                nc.vector.tensor_scalar_mul(Srow[i][:wi, ds(i * 128, vw)],
                                            psS[i][:wi, 0:vw], -1.0)
                dgblk = work.tile([128, 128], BF16, tag="dgblk")
                nc.vector.tensor_scalar_mul(dgblk, ident, dSs[i])
                nc.vector.tensor_add(Srow[i][:, ts(i, 128)],
                                     Srow[i][:, ts(i, 128)], dgblk)
                nc.vector.tensor_sub(zvec[i][:wi, :], cbts[i][:wi, 1:2],
                                     psS[i][:wi, vw:vw + 1])

        # pad fixes for logA / qa (partitions 125:128)
        nc.vector.copy_predicated(Av, pmask8, ones128[:, 0:NCORES])
        nc.vector.copy_predicated(aAv, pmask8, zcol)
        scr8 = work.tile([128, NCORES], F32, tag="scr8")
        logA = work.tile([128, 1], F32, tag="logA")
        nc.scalar.activation(scr8, Av, ACT.Ln, accum_out=logA)
        nc.vector.tensor_mul(scr8, aAv, aAv)
        nc.vector.tensor_mul(scr8, scr8, Winv)
        qa = work.tile([128, 1], F32, tag="qa")
        nc.vector.tensor_reduce(qa, scr8, AX.X, OP.add)

        # ---- block LDL: NS inverses + deferred Chebyshev traces ----
        ns_b = ns_init_coeffs()[1]
        Binv = [work.tile([128, 128], F32, tag=f"Binv{k}", name=f"Binv{k}")
                for k in range(NBLK)]
        Wk = [work.tile([128, SP - (k + 1) * 128], BF16, tag=f"Wk{k}",
                        name=f"Wk{k}") for k in range(NBLK - 1)]
        Wk32 = [work.tile([128, SP - (k + 1) * 128], F32, tag=f"Wk32_{k}",
                          name=f"Wk32_{k}") for k in range(NBLK - 1)]
        trc = work.tile([128, NBLK], F32, tag="trc")
        qtt = work.tile([128, NBLK], F32, tag="qtt")

        with (
            tc.tile_pool(name="ldl", bufs=4) as ldl,
            tc.tile_pool(name="ldl_ps", bufs=4, space=bass.MemorySpace.PSUM) as lps,
        ):
            def cheb_chain(k):
                # trace of log(Bk) via Chebyshev; R += c_j T_j on the PE
                Bk = Srow[k][:, ts(k, 128)]
                bh = ldl.tile([128, 128], BF16, tag=f"bh{k}", name=f"bh{k}")
                nc.vector.tensor_scalar_mul(bh, Bk, 2.0 / (HI - LO))
                nc.vector.tensor_sub(bh, bh, shiftI)
                b2 = ldl.tile([128, 128], BF16, tag=f"b2{k}", name=f"b2{k}")
                nc.vector.tensor_scalar_mul(b2, bh, 2.0)
                R = lps.tile([128, 128], F32, tag="Rps", bufs=4,
                             name=f"Rps{k}")
                nc.tensor.matmul(R, cIh[0], bh, start=True,
                                 stop=(CHEB_DEG == 1))
                tprev, tcur = identB16, bh
                for j in range(2, CHEB_DEG + 1):
                    psc = lps.tile([128, 128], F32, tag="lps")
                    nc.tensor.matmul(psc, b2, tcur, start=True, stop=True)
                    tnext = ldl.tile([128, 128], BF16, tag=f"chT{k}",
                                     name=f"chT{k}_{j}", bufs=3)
                    nc.vector.tensor_sub(tnext, psc, tprev)
                    nc.tensor.matmul(R, cIh[j - 1], tnext,
                                     start=False, stop=(j == CHEB_DEG))
                    tprev, tcur = tcur, tnext
                Rsb = ldl.tile([128, 128], F32, tag="Rsb")
                nc.vector.tensor_mul(Rsb, R, ident)   # keep diagonal only
                nc.vector.tensor_reduce(trc[:, k:k + 1], Rsb, AX.X, OP.add)

            cheb_chain(0)
            for k in range(NBLK):
                Bk = Srow[k][:, ts(k, 128)]
                # NS: X0 = aI + b*B; X <- (2I - X B) X, all iterates sym.
                X = ldl.tile([128, 128], BF16, tag="nsX")
                nc.vector.tensor_scalar_mul(X, Bk, ns_b)
                nc.vector.tensor_add(X, X, nsAI)
                psX = None
                for it in range(NS_ITERS):
                    psP = lps.tile([128, 128], F32, tag="lps")
                    nc.tensor.matmul(psP, X, Bk, start=True, stop=True)
                    Z = ldl.tile([128, 128], BF16, tag="nsZ")
                    nc.vector.tensor_sub(Z, i2, psP)
                    psX = lps.tile([128, 128], F32, tag="lps")
                    nc.tensor.matmul(psX, Z, X, start=True, stop=True)
                    X = ldl.tile([128, 128], BF16, tag="nsX")
                    nc.vector.tensor_copy(X, psX)
                nc.vector.tensor_copy(Binv[k], psX)
                trail = SP - (k + 1) * 128 if k < NBLK - 1 else 0
                if trail:
                    psW = lps.tile([128, 384], F32, tag="lps")
                    nc.tensor.matmul(psW[:, :trail], X,
                                     Srow[k][:, (k + 1) * 128:SP],
                                     start=True, stop=True)
                    nc.vector.tensor_copy(Wk[k], psW[:, :trail])
                    nc.vector.tensor_copy(Wk32[k], psW[:, :trail])
                    for i in range(k + 1, NBLK):
                        uw = SP - 128 * i
                        off = (i - k - 1) * 128
                        psu = lps.tile([128, 384], F32, tag="lps")
                        nc.tensor.matmul(psu[:, :uw], Srow[k][:, ts(i, 128)],
                                         Wk[k][:, ds(off, uw)],
                                         start=True, stop=True)
                        nc.vector.tensor_sub(Srow[i][:, ds(128 * i, uw)],
                                             Srow[i][:, ds(128 * i, uw)],
                                             psu[:, :uw])
                        if i == k + 1:
                            cheb_chain(i)

            # forward substitution: z_i -= (Wk[k] block i)^T z_k
            for k in range(NBLK - 1):
                for i in range(k + 1, NBLK):
                    psz = lps.tile([128, 1], F32, tag="lps")
                    off = (i - k - 1) * 128
                    nc.tensor.matmul(psz, Wk32[k][:, ds(off, 128)], zvec[k],
                                     start=True, stop=True)
                    nc.vector.tensor_sub(zvec[i], zvec[i], psz)
            # quad_t = sum_k z_k^T Binv_k z_k
            for k in range(NBLK):
                psq = lps.tile([128, 1], F32, tag="lps")
                nc.tensor.matmul(psq, Binv[k], zvec[k], start=True, stop=True)
                uk = ldl.tile([128, 1], F32, tag="uk")
                nc.vector.tensor_copy(uk, psq)
                nc.vector.tensor_mul(qtt[:, k:k + 1], zvec[k], uk)

        # ---- final scalar assembly ----
        qtr = work.tile([128, 1], F32, tag="qtr")
        nc.vector.tensor_reduce(qtr, qtt, AX.X, OP.add)
        nc.vector.tensor_copy(smalls_c[:, 0:1], logA)
        nc.vector.tensor_copy(smalls_c[:, 1:2], qa)
        nc.vector.tensor_copy(smalls_c[:, 2:3], qtr)
        nc.vector.tensor_copy(smalls_c[:, 3:3 + NBLK], trc)
        smalls = work.tile([1, 9], F32, tag="smalls")
        ldS = work.tile([1, 1], F32, tag="ldS")
        with tc.tile_pool(name="fin_ps", bufs=1,
                          space=bass.MemorySpace.PSUM) as gps2:
            ps_sm = gps2.tile([128, 9], F32, tag="gps2")
            nc.tensor.matmul(ps_sm[0:1, :], ones128[:, 0:1], smalls_c,
                             start=True, stop=True)
            nc.vector.tensor_copy(smalls, ps_sm[0:1, :])
        nc.vector.tensor_reduce(ldS, smalls[:, 3:3 + NBLK], AX.X, OP.add)
        mtm = smalls[:, 7:8]
        r2g = smalls[:, 8:9]

        fin = work.tile([1, 8], F32, tag="fin")
        # quadK = quad_a + quad_t
        nc.vector.tensor_add(fin[:, 0:1], smalls[:, 1:2], smalls[:, 2:3])
        # mVinvm = (sig2/sig2e) * (mtm - quadK)
        nc.vector.tensor_sub(fin[:, 1:2], mtm, fin[:, 0:1])
        nc.vector.tensor_scalar_mul(fin[:, 1:2], fin[:, 1:2], cst[0:1, 6:7])
        # logdetV = const (incl. c0*SP + pad corr) + sum log A + tr chains
        nc.vector.tensor_add(fin[:, 2:3], smalls[:, 0:1], ldS)
        nc.vector.tensor_scalar(out=fin[:, 2:3], in0=fin[:, 2:3],
                                scalar1=cst[0:1, 4:5], scalar2=None, op0=OP.add)
        # sum_log_pdf = const2 - sum_r2/(2 sig2)
        nc.vector.tensor_scalar(out=fin[:, 3:4], in0=r2g, scalar1=cst[0:1, 7:8],
                                scalar2=cst[0:1, 5:6], op0=OP.mult, op1=OP.add)
        # total = 0.5*(logdetV + mVinvm - mtm + sum_log_pdf)
        nc.vector.tensor_add(fin[:, 4:5], fin[:, 2:3], fin[:, 1:2])
        nc.vector.tensor_sub(fin[:, 4:5], fin[:, 4:5], mtm)
        nc.vector.tensor_add(fin[:, 4:5], fin[:, 4:5], fin[:, 3:4])
        nc.vector.tensor_scalar_mul(fin[:, 4:5], fin[:, 4:5], 0.5)

        nc.sync.dma_start(out_d[:], fin[:, 4:5])

    nc.finalize()
    return nc


def host_consts(sig2e, sig2bs, core):
    import ml_dtypes
    s0, s1 = float(sig2bs[0]), float(sig2bs[1])
    sig2e = float(sig2e)
    sig2 = sig2e + s0 + s1
    cs = cheb_coeffs()
    csb = cs.astype(ml_dtypes.bfloat16).astype(np.float64)
    # exact c0 for all SP rows + remove the 12 pad rows' full cheb-log value
    xpad = (2.0 * PADV - (HI + LO)) / (HI - LO)
    tj, tp, tc_ = 0.0, 1.0, xpad
    chebpad = csb[1] * xpad
    for j in range(2, CHEB_DEG + 1):
        tn = 2.0 * xpad * tc_ - tp
        chebpad += csb[j] * tn
        tp, tc_ = tc_, tn
    chebpad += cs[0]
    c = np.zeros(16, np.float32)
    c[0] = 1.0 / math.sqrt(sig2)
    c[1] = CLIP
    c[2] = sig2e / s0
    c[3] = sig2e / s1
    c[4] = ((N - Q0 - Q1) * math.log(sig2e) + Q0 * math.log(s0)
            + Q1 * math.log(s1) - N * math.log(sig2)
            + SP * cs[0] - (128 - W3) * chebpad)
    c[5] = -0.5 * N * math.log(2.0 * math.pi * sig2)
    c[6] = sig2 / sig2e
    c[7] = -1.0 / (2.0 * sig2)
    c[8] = -CLIP
    c[9] = -float(B0 * core)
    return c


_CACHE = {}


def _get_module(n_cores=NCORES):
    if n_cores not in _CACHE:
        _CACHE[n_cores] = build_module(n_cores)
    return _CACHE[n_cores]


def make_in_maps(inputs, n_cores=NCORES):
    import ml_dtypes
    y_true = np.asarray(inputs["y_true"], np.float32).reshape(N)
    y_pred = np.asarray(inputs["y_pred"], np.float32).reshape(N)
    zi0 = np.asarray(inputs["Z_idx0"]).astype(np.int32).reshape(N)
    zi1 = np.asarray(inputs["Z_idx1"]).astype(np.int32).reshape(N)
    pk = np.concatenate([
        y_true.reshape(NCH, 128).T,
        y_pred.reshape(NCH, 128).T,
        zi0.reshape(NCH, 128).T.view(np.float32),
        zi1.reshape(NCH, 128).T.view(np.float32),
    ], axis=1)
    pk = np.ascontiguousarray(pk)
    cs = cheb_coeffs()
    chebc = cs.astype(ml_dtypes.bfloat16).astype(np.float32)
    maps = []
    for i in range(n_cores):
        c = host_consts(np.asarray(inputs["sig2e"]),
                        np.asarray(inputs["sig2bs"], np.float64), i)
        maps.append({"packed": pk, "consts": c, "chebc": chebc})
    return maps


def kernel(**inputs):
    nc = _get_module(NCORES)
    maps = make_in_maps(inputs, NCORES)
    res = run_bass_kernel_spmd(nc, maps, list(range(NCORES)))
    out = np.asarray(res.results[0]["out"], np.float32).reshape(1, 1)
    return out


# revision 31
# speedup vs baseline: 1.2655x; 1.0360x over previous
"""COPNLL loss kernel for Trainium2 (8 NeuronCores) — v2.

Math: V = (sig2e*I + s0*Z0 Z0^T + s1*Z1 Z1^T)/sig2 with Z0 (4096x1000),
Z1 (4096x500) one-hot. Woodbury reduces logdet(V) and m^T V^-1 m to the
500x500 Schur complement S = D1 - C^T diag(1/A) C with C = Z0^T Z1,
A = sig2e/s0 + counts0, D1 = sig2e/s1*I + diag(counts1).

Device plan (SPMD on 8 cores), column-sharded phase A:
  Each core processes ALL 4096 rows but only its 125 level-0 groups.
  Per 128-row chunk, ONE accumulating matmul with
    stationary st = [1 | m | r | oh0_125]  (exactly 128 cols)
    moving     rh = [oh1_500 | 1 | m | r]  (503 cols)
  yields C_block, counts0_block, a_block (rows 3:128) AND the global
  counts1, b, N, sum m, mtm, sum r^2 (rows 0:3) — no reduction needed.
  The ONLY collective is a single 64KB/rank int8 AllGather of
  [C_block int8 | counts0,a bitcast-f32]; counts1/b/mtm/r2 are computed
  fully locally on every core. A tiny warm-up AllGather posted first
  absorbs the CC-stream entry barrier during compute.
Phase C (redundant on all cores): upper-triangle S assembly (the t-vector
  rides as an appended moving column), block LDL with 4-iter Newton-Schulz
  inverses (quadratic init), Chebyshev trace of log (deg 7, bf16 coeffs,
  c0 + pad correction folded into a host constant).
"""

import math
import sys
import types

import numpy as np

import concourse.bass as bass
import concourse.bacc as bacc
import concourse.mybir as mybir
from concourse.bass import ds, ts
from concourse.bass_utils import run_bass_kernel_spmd
from concourse.masks import make_identity
from concourse.tile import TileContext


def _ensure_axon_hooks():
    """bass_utils imports antenv.axon_hooks when tracing; this image's antenv
    lacks it. Provide a shim (with the real ctypes NTFF hook when available)
    so trace=True/BASS_TRACE never crashes the kernel."""
    try:
        import antenv.axon_hooks  # noqa: F401
        return
    except ImportError:
        pass
    try:
        import trn_agent_boot.trn_boot as tb
        hook = tb._ntff_profile_via_ctypes("/opt/axon/libaxon_pjrt.so")
    except Exception:
        hook = None
    mod = types.ModuleType("antenv.axon_hooks")
    mod._hook = hook
    mod.get_axon_ntff_profile_hook = lambda: mod._hook

    def _set(h):
        mod._hook = h

    mod.set_axon_ntff_profile_hook = _set
    sys.modules["antenv.axon_hooks"] = mod
    try:
        import antenv
        antenv.axon_hooks = mod
    except ImportError:
        pass
    try:
        import concourse.bass_utils as bu
        _orig_upload = bu.upload_artifacts

        def _safe_upload(tmpdir):
            try:
                return _orig_upload(tmpdir)
            except Exception:
                return f"local:{tmpdir}"

        bu.upload_artifacts = _safe_upload
    except Exception:
        pass


_ensure_axon_hooks()

N = 4096
NCORES = 8
NCH = 32                   # 128-row chunks (all rows on every core)
NGRP = 4                   # chunk groups for build/matmul overlap
GSZ = NCH // NGRP          # 8 chunks per group
Q0 = 1000
B0 = Q0 // NCORES          # 125 level-0 groups per core
Q1 = 500
FR = Q1 + 2                # rh matmul width: [Z1 | 1 | m]
PAYW = 512                 # payload row bytes (int8)
SP = 512                   # padded S size
NBLK = SP // 128           # 4
W3 = Q1 - 3 * 128          # 116: valid width of the last S block
PADV = 4.0                 # pad diagonal value (mid-spectrum)
LO, HI = 1.4, 18.0         # eigenvalue bounds for NS init + Chebyshev
NS_ITERS = 3
CHEB_DEG = 7
NCOEF = CHEB_DEG + 1
CLIP = 4.2648907939226017  # sqrt(2)*erfinv(1-2e-5)
WARMUP = False

F32 = mybir.dt.float32
BF16 = mybir.dt.bfloat16
I8 = mybir.dt.int8
I32 = mybir.dt.int32
AX = mybir.AxisListType
OP = mybir.AluOpType
ACT = mybir.ActivationFunctionType

AG_IN = 128 * PAYW             # int8 bytes per rank
AG_OUT = NCORES * AG_IN


def cheb_coeffs(lo=LO, hi=HI, deg=CHEB_DEG):
    K = 4000
    th = (np.arange(K) + 0.5) * np.pi / K
    xk = np.cos(th)
    fk = np.log((hi - lo) / 2.0 * xk + (hi + lo) / 2.0)
    cs = np.array([2.0 / K * np.sum(fk * np.cos(j * th)) for j in range(deg + 1)])
    cs[0] *= 0.5
    return cs


def ns_init_coeffs(lo=LO, hi=HI):
    # X0 = a*I + b*B: minimax linear init for NS (equioscillating residual)
    m = (lo + hi) / 2.0
    s = (hi - lo) / 2.0
    b = 1.0 / (s * s / 2.0 - m * m)
    a = -2.0 * b * m
    return a, b


def _diag_fill(nc, tile_ap, value):
    nc.gpsimd.memset(tile_ap, 0.0)
    nc.gpsimd.affine_select(out=tile_ap, in_=tile_ap, compare_op=OP.not_equal,
                            fill=value, base=0, pattern=[[-1, 128]],
                            channel_multiplier=1)


def build_module(n_cores=NCORES, warmup=WARMUP):
    nc = bacc.Bacc(num_devices=n_cores)
    pk_d = nc.declare_dram_parameter("packed", [128, 4 * NCH], F32,
                                     isOutput=False)
    cst_d = nc.declare_dram_parameter("consts", [16], F32, isOutput=False)
    out_d = nc.declare_dram_parameter("out", [1, 1], F32, isOutput=True)

    ag_in = nc.dram_tensor("ag_in", [AG_IN], I8)
    ag_out = nc.dram_tensor("ag_out", [AG_OUT], I8, addr_space="Shared")
    warm_in = nc.dram_tensor("warm_in", [64], F32)
    warm_out = nc.dram_tensor("warm_out", [64 * n_cores], F32,
                              addr_space="Shared")
    rg = [list(range(n_cores))]

    with TileContext(nc) as tc, \
         tc.tile_pool(name="consts", bufs=1) as consts, \
         tc.tile_pool(name="work", bufs=1) as work:

        # ---- warm-up collective: absorbs the CC entry barrier early ----
        if warmup and n_cores > 1:
            nc.gpsimd.collective_compute(
                "AllGather", OP.bypass, replica_groups=rg,
                ins=[warm_in[:]], outs=[warm_out[:]],
            )

        # ---- constants ----
        ident = consts.tile([128, 128], F32, tag="ident")
        make_identity(nc, ident)
        identB16 = consts.tile([128, 128], BF16, tag="identB16")
        nc.vector.tensor_copy(identB16, ident)
        ones128 = consts.tile([128, 128], F32, tag="ones128")
        nc.vector.memset(ones128, 1.0)

        cst_row = consts.tile([1, 16], F32, tag="cst_row")
        nc.sync.dma_start(cst_row, cst_d[:].rearrange("(p x) -> p x", p=1))
        cst = consts.tile([128, 16], F32, tag="cst")
        with tc.tile_pool(name="setup_ps", bufs=2,
                          space=bass.MemorySpace.PSUM) as gps0:
            ps_b = gps0.tile([128, 16], F32, tag="gps0")
            nc.tensor.matmul(ps_b, ones128[0:1, :], cst_row,
                             start=True, stop=True)
            nc.vector.tensor_copy(cst, ps_b)

        # iotas for the one-hot compares (fp16: ints < 2048 exact, 2x DVE)
        F16 = mybir.dt.float16
        iota0i = work.tile([128, B0], I32, tag="iota0i")
        nc.gpsimd.iota(iota0i, pattern=[[1, B0]], base=0, channel_multiplier=0)
        iota0 = work.tile([128, B0], F32, tag="iota0")
        nc.vector.tensor_copy(iota0, iota0i)
        iota1i = work.tile([128, Q1], I32, tag="iota1i")
        nc.gpsimd.iota(iota1i, pattern=[[1, Q1]], base=0, channel_multiplier=0)
        iota1 = work.tile([128, Q1], F32, tag="iota1")
        nc.vector.tensor_copy(iota1, iota1i)
        # partition index (for pad masks on partitions 0..2)
        iotaPi = work.tile([128, 1], I32, tag="iotaPi")
        nc.gpsimd.iota(iotaPi, pattern=[[1, 1]], base=0, channel_multiplier=1)
        iotaP = work.tile([128, 1], F32, tag="iotaP")
        nc.vector.tensor_copy(iotaP, iotaPi)

        # ---- inputs -> m, resid ----
        packed = work.tile([128, 4 * NCH], F32, tag="packed")
        nc.sync.dma_start(packed, pk_d[:])
        yt = packed[:, 0:NCH]
        yp = packed[:, NCH:2 * NCH]
        idx0f = work.tile([128, NCH], F32, tag="idx0f")
        nc.vector.tensor_copy(idx0f, packed[:, 2 * NCH:3 * NCH].bitcast(I32))
        # idx0 shifted into this core's block: idx0s = idx0 - 125*core
        idx0 = work.tile([128, NCH], F32, tag="idx0")
        nc.vector.tensor_scalar(out=idx0, in0=idx0f, scalar1=cst[:, 9:10],
                                scalar2=None, op0=OP.add)
        idx1 = work.tile([128, NCH], F32, tag="idx1")
        nc.vector.tensor_copy(idx1, packed[:, 3 * NCH:4 * NCH].bitcast(I32))
        resid = work.tile([128, NCH], F32, tag="resid")
        nc.vector.tensor_sub(resid, yt, yp)
        mvec = work.tile([128, NCH], F32, tag="mvec")
        nc.vector.tensor_scalar(out=mvec, in0=resid, scalar1=cst[:, 0:1],
                                scalar2=cst[:, 1:2], op0=OP.mult, op1=OP.min)
        nc.vector.tensor_scalar(out=mvec, in0=mvec, scalar1=cst[:, 8:9],
                                scalar2=None, op0=OP.max)
        smalls_c = work.tile([128, 9], F32, tag="smalls_c")
        nc.vector.memset(smalls_c, 0.0)
        scrN = work.tile([128, NCH], F32, tag="scrN")
        nc.vector.tensor_mul(scrN, mvec, mvec)
        nc.vector.tensor_reduce(smalls_c[:, 7:8], scrN, AX.X, OP.add)
        nc.vector.tensor_mul(scrN, resid, resid)
        nc.vector.tensor_reduce(smalls_c[:, 8:9], scrN, AX.X, OP.add)

        # ---- phase A: ONE matmul per chunk, grouped for overlap ----
        # st cols: 0:125 = oh0 (aligned writes), 125 = ones, 126 = m, 127 = 0
        # rh cols: 0:500 = oh1 (aligned; chunk stride padded to 512),
        #          500 = ones, 501 = m
        # psC: rows 0:125 = C|counts0|a, row 125 = counts1|N|sum m,
        #      row 126 = b|sum m|mtm, row 127 = 0
        STg = [work.tile([128, GSZ, 128], BF16, tag=f"STg{g}", name=f"STg{g}")
               for g in range(NGRP)]
        RHg = [work.tile([128, GSZ, 512], BF16, tag=f"RHg{g}", name=f"RHg{g}")
               for g in range(NGRP)]
        for g in range(NGRP):
            sl = slice(g * GSZ, (g + 1) * GSZ)
            nc.gpsimd.memset(STg[g][:, :, 125], 1.0)
            nc.gpsimd.memset(STg[g][:, :, 127], 0.0)
            nc.vector.tensor_copy(STg[g][:, :, 126], mvec[:, sl])
            nc.gpsimd.memset(RHg[g][:, :, Q1], 1.0)
            nc.vector.tensor_copy(RHg[g][:, :, Q1 + 1], mvec[:, sl])
            for cc in range(GSZ):
                c = g * GSZ + cc
                nc.vector.tensor_scalar(out=STg[g][:, cc, 0:B0], in0=iota0,
                                        scalar1=idx0[:, c:c + 1],
                                        scalar2=None, op0=OP.is_equal)
                nc.vector.tensor_scalar(out=RHg[g][:, cc, 0:Q1], in0=iota1,
                                        scalar1=idx1[:, c:c + 1],
                                        scalar2=None, op0=OP.is_equal)

        pay = work.tile([128, PAYW], I8, tag="pay")
        g1s = work.tile([128, Q1], F32, tag="g1s")
        cf32 = work.tile([128, Q1], F32, tag="cf32")
        with tc.tile_pool(name="phA_ps", bufs=1,
                          space=bass.MemorySpace.PSUM) as pps:
            psC = pps.tile([128, FR], F32, tag="psC")
            for g in range(NGRP):
                for cc in range(GSZ):
                    c = g * GSZ + cc
                    nc.tensor.matmul(psC, STg[g][:, cc, :],
                                     RHg[g][:, cc, 0:FR],
                                     start=(c == 0), stop=(c == NCH - 1))
            # extract: C block -> int8 payload; counts0|a bitcast as f32
            nc.vector.tensor_copy(pay[:, 0:Q1], psC[:, 0:Q1])
            nc.vector.tensor_copy(pay[:, Q1:Q1 + 8].bitcast(F32),
                                  psC[:, Q1:Q1 + 2])
            # counts1 | b live on partitions 125:127: bounce to f32 SBUF,
            # then partition-shift to 0:2 via SBUF->SBUF DMA
            nc.vector.tensor_copy(cf32, psC[:, 0:Q1])
        nc.sync.dma_start(g1s[0:2, :], cf32[125:127, :])

        # ---- the single collective: AllGather of [C | counts0 | a] ----
        nc.sync.dma_start(ag_in[:].rearrange("(p f) -> p f", p=128), pay)
        if n_cores > 1:
            nc.gpsimd.collective_compute(
                "AllGather", OP.bypass, replica_groups=rg,
                ins=[ag_in[:]], outs=[ag_out[:]],
            )
        else:
            nc.sync.dma_start(ag_out[:], ag_in[:])

        # ---- phase C constants (fill the collective wait) ----
        i2 = consts.tile([128, 128], F32, tag="i2")              # 2*I
        _diag_fill(nc, i2, 2.0)
        shiftI = consts.tile([128, 128], F32, tag="shiftI")      # Cheb shift
        _diag_fill(nc, shiftI, (HI + LO) / (HI - LO))
        nsAI = consts.tile([128, 128], F32, tag="nsAI")          # NS init aI
        _diag_fill(nc, nsAI, ns_init_coeffs()[0])

        # c1 (counts1) / b per S-block as partition vectors via PE transpose
        cbts = []
        dSs = []
        with tc.tile_pool(name="tr_ps", bufs=2,
                          space=bass.MemorySpace.PSUM) as tps:
            for i in range(NBLK):
                wi = 128 if i < NBLK - 1 else W3
                psT = tps.tile([128, 2], F32, tag="pst")
                nc.tensor.transpose(psT[:wi, :], g1s[0:2, ds(i * 128, wi)],
                                    ident[0:2, 0:2])
                cbt = work.tile([128, 2], F32, tag=f"cb{i}", name=f"cb{i}")
                nc.vector.memset(cbt, 0.0)
                nc.vector.tensor_copy(cbt[:wi, :], psT[:wi, :])
                cbts.append(cbt)
                dS = work.tile([128, 1], F32, tag=f"dS{i}", name=f"dS{i}")
                nc.vector.tensor_scalar(out=dS, in0=cbt[:, 0:1],
                                        scalar1=cst[:, 3:4],
                                        scalar2=None, op0=OP.add)
                if i == NBLK - 1:
                    pm3 = work.tile([128, 1], mybir.dt.uint32, tag="pm3")
                    nc.vector.tensor_scalar(out=pm3, in0=iotaP,
                                            scalar1=float(W3) - 0.5,
                                            scalar2=None, op0=OP.is_gt)
                    padv = work.tile([128, 1], F32, tag="padv")
                    nc.vector.memset(padv, PADV)
                    nc.vector.copy_predicated(dS, pm3, padv)
                dSs.append(dS)

        # ---- unpack the AllGather + S assembly, pipelined per tile ----
        # valid level-0 rows on partitions 0:125; pads 125:128 are killed by
        # zeroing Winv there (stationary Cw pad rows become exactly 0)
        pmaskP = work.tile([128, 1], mybir.dt.uint32, tag="pmaskP")
        nc.vector.tensor_scalar(out=pmaskP, in0=iotaP, scalar1=float(B0) - 0.5,
                                scalar2=None, op0=OP.is_gt)
        iotaP8 = work.tile([128, NCORES], F32, tag="iotaP8")
        nc.vector.tensor_scalar_mul(iotaP8, ones128[:, 0:NCORES], iotaP)
        pmask8 = work.tile([128, NCORES], mybir.dt.uint32, tag="pmask8")
        nc.vector.tensor_scalar(out=pmask8, in0=iotaP8, scalar1=float(B0) - 0.5,
                                scalar2=None, op0=OP.is_gt)
        zcol = work.tile([128, NCORES], F32, tag="zcol")
        nc.vector.memset(zcol, 0.0)

        CT8 = [work.tile([128, PAYW], I8, tag=f"CT8_{t % 2}", name=f"CT8_{t}")
               for t in range(NCORES)]
        G0 = work.tile([128, NCORES, 512], BF16, tag="G0")   # [C | a | pad]
        Cw = work.tile([128, NCORES, 512], BF16, tag="Cw")
        Av = work.tile([128, NCORES], F32, tag="Av")
        Winv = work.tile([128, NCORES], F32, tag="Winv")
        aAv = work.tile([128, NCORES], F32, tag="aAv")
        Srow = [work.tile([128, SP], BF16, tag=f"Srow{i}", name=f"Srow{i}")
                for i in range(NBLK)]
        zvec = [work.tile([128, 1], F32, tag=f"z{i}", name=f"z{i}")
                for i in range(NBLK)]
        for i in range(NBLK):
            nc.vector.memset(Srow[i], 0.0)
            nc.vector.memset(zvec[i], 0.0)

        with tc.tile_pool(name="sasm_ps", bufs=1,
                          space=bass.MemorySpace.PSUM) as sps:
            psS = [sps.tile([128, Q1 + 1 - 128 * i], F32, tag=f"psS{i}",
                            name=f"psS{i}") for i in range(NBLK)]
            for t in range(NCORES):
                slot = ag_out[t * AG_IN:(t + 1) * AG_IN]
                nc.sync.dma_start(CT8[t], slot.rearrange("(p f) -> p f", p=128))
                nc.vector.tensor_copy(G0[:, t, 0:Q1], CT8[t][:, 0:Q1])
                cnA = CT8[t][:, Q1:Q1 + 8].bitcast(F32)   # [128, 2]
                nc.vector.tensor_scalar(out=Av[:, t:t + 1], in0=cnA[:, 0:1],
                                        scalar1=cst[:, 2:3], scalar2=None,
                                        op0=OP.add)
                nc.vector.tensor_copy(aAv[:, t:t + 1], cnA[:, 1:2])
                nc.vector.tensor_copy(G0[:, t, Q1:Q1 + 1], aAv[:, t:t + 1])
                nc.vector.reciprocal(Winv[:, t:t + 1], Av[:, t:t + 1])
                nc.vector.copy_predicated(Winv[:, t:t + 1], pmaskP,
                                          zcol[:, 0:1])
                nc.vector.tensor_scalar_mul(Cw[:, t, 0:Q1], G0[:, t, 0:Q1],
                                            Winv[:, t:t + 1])
                for i in range(NBLK):
                    wi = 128 if i < NBLK - 1 else W3
                    mw = Q1 + 1 - 128 * i
                    nc.tensor.matmul(psS[i][:wi, :],
                                     Cw[:, t, ds(i * 128, wi)],
                                     G0[:, t, ds(i * 128, mw)],
                                     start=(t == 0), stop=(t == NCORES - 1))
            # S rows (upper triangle) + diagonal; z = b - C^T (a/A)
            for i in range(NBLK):
                wi = 128 if i < NBLK - 1 else W3
                vw = Q1 - 128 * i
                nc.vector.tensor_scalar_mul(Srow[i][:wi, ds(i * 128, vw)],
                                            psS[i][:wi, 0:vw], -1.0)
                dgblk = work.tile([128, 128], BF16, tag="dgblk")
                nc.vector.tensor_scalar_mul(dgblk, ident, dSs[i])
                nc.vector.tensor_add(Srow[i][:, ts(i, 128)],
                                     Srow[i][:, ts(i, 128)], dgblk)
                nc.vector.tensor_sub(zvec[i][:wi, :], cbts[i][:wi, 1:2],
                                     psS[i][:wi, vw:vw + 1])

        # pad fixes for logA / qa (partitions 125:128)
        nc.vector.copy_predicated(Av, pmask8, ones128[:, 0:NCORES])
        nc.vector.copy_predicated(aAv, pmask8, zcol)
        scr8 = work.tile([128, NCORES], F32, tag="scr8")
        logA = work.tile([128, 1], F32, tag="logA")
        nc.scalar.activation(scr8, Av, ACT.Ln, accum_out=logA)
        nc.vector.tensor_mul(scr8, aAv, aAv)
        nc.vector.tensor_mul(scr8, scr8, Winv)
        qa = work.tile([128, 1], F32, tag="qa")
        nc.vector.tensor_reduce(qa, scr8, AX.X, OP.add)

        # ---- block LDL: NS inverses + deferred Chebyshev traces ----
        ns_b = ns_init_coeffs()[1]
        Binv = [work.tile([128, 128], F32, tag=f"Binv{k}", name=f"Binv{k}")
                for k in range(NBLK)]
        Wk = [work.tile([128, SP - (k + 1) * 128], BF16, tag=f"Wk{k}",
                        name=f"Wk{k}") for k in range(NBLK - 1)]
        Wk32 = [work.tile([128, SP - (k + 1) * 128], F32, tag=f"Wk32_{k}",
                          name=f"Wk32_{k}") for k in range(NBLK - 1)]
        trc = work.tile([128, NBLK], F32, tag="trc")
        qtt = work.tile([128, NBLK], F32, tag="qtt")

        with (
            tc.tile_pool(name="ldl", bufs=4) as ldl,
            tc.tile_pool(name="ldl_ps", bufs=4, space=bass.MemorySpace.PSUM) as lps,
        ):
            csf = [float(x) for x in cheb_coeffs()]
            trcd = [work.tile([128, CHEB_DEG], F32, tag=f"trcd{k}",
                              name=f"trcd{k}") for k in range(NBLK)]

            def cheb_chain(k):
                # trace of log(Bk) via Chebyshev: each degree's c_j*diag
                # contribution lands in its own column of trcd[k] via a fused
                # multiply-by-identity + scaled reduce (c_j is the
                # compile-time scale); one row-reduce at the end.
                Bk = Srow[k][:, ts(k, 128)]
                bh = ldl.tile([128, 128], BF16, tag=f"bh{k}", name=f"bh{k}")
                nc.vector.tensor_scalar_mul(bh, Bk, 2.0 / (HI - LO))
                nc.vector.tensor_sub(bh, bh, shiftI)
                b2 = ldl.tile([128, 128], BF16, tag=f"b2{k}", name=f"b2{k}")
                nc.vector.tensor_scalar_mul(b2, bh, 2.0)
                scrD = ldl.tile([128, 128], BF16, tag="ttrscr")
                nc.vector.tensor_mul(scrD, bh, identB16)
                scr2 = ldl.tile([128, 128], F32, tag="ttrscr2")
                nc.scalar.activation(scr2, scrD, ACT.Copy, scale=csf[1],
                                     accum_out=trcd[k][:, 0:1])
                tprev, tcur = identB16, bh
                for j in range(2, CHEB_DEG + 1):
                    psc = lps.tile([128, 128], F32, tag="lps")
                    nc.tensor.matmul(psc, b2, tcur, start=True, stop=True)
                    tnext = ldl.tile([128, 128], BF16, tag=f"chT{k}",
                                     name=f"chT{k}_{j}", bufs=3)
                    nc.vector.tensor_sub(tnext, psc, tprev)
                    scrD = ldl.tile([128, 128], BF16, tag="ttrscr")
                    nc.vector.tensor_mul(scrD, tnext, identB16)
                    scr2 = ldl.tile([128, 128], F32, tag="ttrscr2")
                    nc.scalar.activation(scr2, scrD, ACT.Copy, scale=csf[j],
                                         accum_out=trcd[k][:, j - 1:j])
                    tprev, tcur = tcur, tnext
                nc.vector.tensor_reduce(trc[:, k:k + 1], trcd[k], AX.X, OP.add)

            cheb_chain(0)
            for k in range(NBLK):
                Bk = Srow[k][:, ts(k, 128)]
                # NS: X0 = aI + b*B; X <- (2I - X B) X, all iterates sym.
                X = ldl.tile([128, 128], BF16, tag="nsX")
                nc.vector.tensor_scalar_mul(X, Bk, ns_b)
                nc.vector.tensor_add(X, X, nsAI)
                psX = None
                for it in range(NS_ITERS):
                    psP = lps.tile([128, 128], F32, tag="lps")
                    nc.tensor.matmul(psP, X, Bk, start=True, stop=True)
                    Z = ldl.tile([128, 128], BF16, tag="nsZ")
                    nc.vector.tensor_sub(Z, i2, psP)
                    psX = lps.tile([128, 128], F32, tag="lps")
                    nc.tensor.matmul(psX, Z, X, start=True, stop=True)
                    X = ldl.tile([128, 128], BF16, tag="nsX")
                    nc.vector.tensor_copy(X, psX)
                nc.vector.tensor_copy(Binv[k], psX)
                trail = SP - (k + 1) * 128 if k < NBLK - 1 else 0
                if trail:
                    psW = lps.tile([128, 384], F32, tag="lps")
                    nc.tensor.matmul(psW[:, :trail], X,
                                     Srow[k][:, (k + 1) * 128:SP],
                                     start=True, stop=True)
                    nc.vector.tensor_copy(Wk[k], psW[:, :trail])
                    nc.vector.tensor_copy(Wk32[k], psW[:, :trail])
                    for i in range(k + 1, NBLK):
                        uw = SP - 128 * i
                        off = (i - k - 1) * 128
                        psu = lps.tile([128, 384], F32, tag="lps")
                        nc.tensor.matmul(psu[:, :uw], Srow[k][:, ts(i, 128)],
                                         Wk[k][:, ds(off, uw)],
                                         start=True, stop=True)
                        nc.vector.tensor_sub(Srow[i][:, ds(128 * i, uw)],
                                             Srow[i][:, ds(128 * i, uw)],
                                             psu[:, :uw])
                        if i == k + 1:
                            cheb_chain(i)

            # forward substitution: z_i -= (Wk[k] block i)^T z_k
            for k in range(NBLK - 1):
                for i in range(k + 1, NBLK):
                    psz = lps.tile([128, 1], F32, tag="lps")
                    off = (i - k - 1) * 128
                    nc.tensor.matmul(psz, Wk32[k][:, ds(off, 128)], zvec[k],
                                     start=True, stop=True)
                    nc.vector.tensor_sub(zvec[i], zvec[i], psz)
            # quad_t = sum_k z_k^T Binv_k z_k
            for k in range(NBLK):
                psq = lps.tile([128, 1], F32, tag="lps")
                nc.tensor.matmul(psq, Binv[k], zvec[k], start=True, stop=True)
                uk = ldl.tile([128, 1], F32, tag="uk")
                nc.vector.tensor_copy(uk, psq)
                nc.vector.tensor_mul(qtt[:, k:k + 1], zvec[k], uk)

        # ---- final scalar assembly ----
        qtr = work.tile([128, 1], F32, tag="qtr")
        nc.vector.tensor_reduce(qtr, qtt, AX.X, OP.add)
        nc.vector.tensor_copy(smalls_c[:, 0:1], logA)
        nc.vector.tensor_copy(smalls_c[:, 1:2], qa)
        nc.vector.tensor_copy(smalls_c[:, 2:3], qtr)
        nc.vector.tensor_copy(smalls_c[:, 3:3 + NBLK], trc)
        smalls = work.tile([1, 9], F32, tag="smalls")
        ldS = work.tile([1, 1], F32, tag="ldS")
        with tc.tile_pool(name="fin_ps", bufs=1,
                          space=bass.MemorySpace.PSUM) as gps2:
            ps_sm = gps2.tile([128, 9], F32, tag="gps2")
            nc.tensor.matmul(ps_sm[0:1, :], ones128[:, 0:1], smalls_c,
                             start=True, stop=True)
            nc.vector.tensor_copy(smalls, ps_sm[0:1, :])
        nc.vector.tensor_reduce(ldS, smalls[:, 3:3 + NBLK], AX.X, OP.add)
        mtm = smalls[:, 7:8]
        r2g = smalls[:, 8:9]

        fin = work.tile([1, 8], F32, tag="fin")
        # quadK = quad_a + quad_t
        nc.vector.tensor_add(fin[:, 0:1], smalls[:, 1:2], smalls[:, 2:3])
        # mVinvm = (sig2/sig2e) * (mtm - quadK)
        nc.vector.tensor_sub(fin[:, 1:2], mtm, fin[:, 0:1])
        nc.vector.tensor_scalar_mul(fin[:, 1:2], fin[:, 1:2], cst[0:1, 6:7])
        # logdetV = const (incl. c0*SP + pad corr) + sum log A + tr chains
        nc.vector.tensor_add(fin[:, 2:3], smalls[:, 0:1], ldS)
        nc.vector.tensor_scalar(out=fin[:, 2:3], in0=fin[:, 2:3],
                                scalar1=cst[0:1, 4:5], scalar2=None, op0=OP.add)
        # sum_log_pdf = const2 - sum_r2/(2 sig2)
        nc.vector.tensor_scalar(out=fin[:, 3:4], in0=r2g, scalar1=cst[0:1, 7:8],
                                scalar2=cst[0:1, 5:6], op0=OP.mult, op1=OP.add)
        # total = 0.5*(logdetV + mVinvm - mtm + sum_log_pdf)
        nc.vector.tensor_add(fin[:, 4:5], fin[:, 2:3], fin[:, 1:2])
        nc.vector.tensor_sub(fin[:, 4:5], fin[:, 4:5], mtm)
        nc.vector.tensor_add(fin[:, 4:5], fin[:, 4:5], fin[:, 3:4])
        nc.vector.tensor_scalar_mul(fin[:, 4:5], fin[:, 4:5], 0.5)

        nc.sync.dma_start(out_d[:], fin[:, 4:5])

    nc.finalize()
    return nc


def host_consts(sig2e, sig2bs, core):
    s0, s1 = float(sig2bs[0]), float(sig2bs[1])
    sig2e = float(sig2e)
    sig2 = sig2e + s0 + s1
    cs = cheb_coeffs()
    # exact c0 for all SP rows + remove the 12 pad rows' full cheb-log value
    xpad = (2.0 * PADV - (HI + LO)) / (HI - LO)
    tp, tc_ = 1.0, xpad
    chebpad = float(np.float32(cs[1])) * xpad
    for j in range(2, CHEB_DEG + 1):
        tn = 2.0 * xpad * tc_ - tp
        chebpad += float(np.float32(cs[j])) * tn
        tp, tc_ = tc_, tn
    chebpad += cs[0]
    c = np.zeros(16, np.float32)
    c[0] = 1.0 / math.sqrt(sig2)
    c[1] = CLIP
    c[2] = sig2e / s0
    c[3] = sig2e / s1
    c[4] = ((N - Q0 - Q1) * math.log(sig2e) + Q0 * math.log(s0)
            + Q1 * math.log(s1) - N * math.log(sig2)
            + SP * cs[0] - (128 - W3) * chebpad)
    c[5] = -0.5 * N * math.log(2.0 * math.pi * sig2)
    c[6] = sig2 / sig2e
    c[7] = -1.0 / (2.0 * sig2)
    c[8] = -CLIP
    c[9] = -float(B0 * core)
    return c


_CACHE = {}


def _get_module(n_cores=NCORES):
    if n_cores not in _CACHE:
        _CACHE[n_cores] = build_module(n_cores)
    return _CACHE[n_cores]


def make_in_maps(inputs, n_cores=NCORES):
    y_true = np.asarray(inputs["y_true"], np.float32).reshape(N)
    y_pred = np.asarray(inputs["y_pred"], np.float32).reshape(N)
    zi0 = np.asarray(inputs["Z_idx0"]).astype(np.int32).reshape(N)
    zi1 = np.asarray(inputs["Z_idx1"]).astype(np.int32).reshape(N)
    pk = np.concatenate([
        y_true.reshape(NCH, 128).T,
        y_pred.reshape(NCH, 128).T,
        zi0.reshape(NCH, 128).T.view(np.float32),
        zi1.reshape(NCH, 128).T.view(np.float32),
    ], axis=1)
    pk = np.ascontiguousarray(pk)
    maps = []
    for i in range(n_cores):
        c = host_consts(np.asarray(inputs["sig2e"]),
                        np.asarray(inputs["sig2bs"], np.float64), i)
        maps.append({"packed": pk, "consts": c})
    return maps


def kernel(**inputs):
    nc = _get_module(NCORES)
    maps = make_in_maps(inputs, NCORES)
    res = run_bass_kernel_spmd(nc, maps, list(range(NCORES)))
    out = np.asarray(res.results[0]["out"], np.float32).reshape(1, 1)
    return out
